# revision 31
# baseline (speedup 1.0000x reference)
"""BloomEmbed kernel for 8 Trainium2 NeuronCores.

Sharding: data-parallel over tokens - each of the 8 cores gets 8192 of the
65536 tokens plus a full replica of the (1/8-pre-scaled, fp16) embedding
table, so no collectives are needed. The Mueller hash runs on host (exact
int64 math); indices are packed on host into int16 gather lists.

Device pipeline (per core), built on the `dma_gather` GPSIMD ucode
(InstDMAGatherAnt) spread across 4 SWDGE queues. Measured on this runtime:
one queue sustains ~9ns/descriptor (ring drain-wait serializes with
generation); rotating calls over 4 queues overlaps them for ~2.7ns/desc.
Plain indirect DMA (the old kernel) is pinned to one queue and 128
descriptors/instruction, which is why it was stuck at ~760us.

  R1 (bucketed gather):  dma_gather reaches 32767 * 256B from a base, so
      the 1M-row fp16 table is covered by 31 windows. One call per
      (section, window) gathers that section's rows from the window (lists
      padded to a uniform CAP with duplicate indices so the program shape
      is data-independent); the sync engine stores each staged tile to an
      HBM scratch at fixed positions. Calls are capped at 1024 indices -
      the SWDGE descriptor ring is ucode-fixed at 1024 entries and a
      larger call wedges the device.
  R2 (token-ordered regather): 64 dma_gather calls of 1024 read the
      scratch in token order, positions arranged so a token's 8 probe rows
      land in one partition at 8 consecutive 256B columns. R1 calls are
      emitted section-major so R2 of section s overlaps R1 of section s+1.
  Reduce: 7 strided DVE adds per chunk sum the probes (table pre-scaled by
      1/8 so the sum is the mean); fp16 results DMA out via the scalar
      engine.

Cross-queue completion is NOT program-ordered, so each queue gets its own
completion semaphore bank (s_r1q[4] / s_g2q[4]); within a queue,
completions are FIFO. Staging and R2 buffers rotate 4-deep so all four
queues stay occupied.

The HBM bounce exists because dma_gather's output position is forced by
list order: bucket-ordered rows cannot feed a fixed-stride reduction
directly, and on-chip reorder primitives (indirect_copy/ap_gather) share
indices across 16-partition groups, which cannot express the per-partition
permutation.
"""

import sys

if "/opt/trn_rl_repo" not in sys.path:
    sys.path.insert(0, "/opt/trn_rl_repo")

import numpy as np

NUM = 1_000_000
DIM = 128
K = 8
B, S = 32, 2048
NCORES = 8
T = B * S  # 65536
T_CORE = T // NCORES  # 8192
P = 128

NSEC = 4  # token sections per core
SEC_TOK = T_CORE // NSEC  # 2048 tokens
SEC_ROWS = SEC_TOK * K  # 16384 gathered rows per section
NBKT = 31  # 32768-row windows covering 1M rows
BKT_ROWS = 32768  # int16 index reach at 256B stride
W_PAD_ROWS = NBKT * BKT_ROWS  # 1015808
NCHUNK = 64  # R2 chunks (16 per section)
CH_IDX = 1024  # rows per R2 chunk (SWDGE ring cap)
CH_TOK = CH_IDX // K  # 128 tokens
NQ = 4  # SWDGE queues
NBUF = 8  # staging/R2 buffer rotation depth (2 calls in flight per queue)

_NC_CACHE = {}


def _mueller_hash(t):
    t = (t >> 16 ^ t) * np.int64(73244475)
    t = (t >> 16 ^ t) * np.int64(73244475)
    t = t >> 16 ^ t
    return t


def _wrap16(lst):
    """int16 list (len % 16 == 0) -> [128, len/16] wrapped+replicated tile."""
    w = lst.reshape(-1, 16).T  # [16, cols]
    return np.tile(w, (8, 1))


def _pack_core(idx24):
    secs = idx24.reshape(NSEC, SEC_TOK * K)  # entry e = u*K + q
    b_all = (secs >> 15).astype(np.int32)
    loc_all = (secs & 32767).astype(np.int16)
    counts = np.zeros((NSEC, NBKT), dtype=np.int64)
    orders = []
    for s in range(NSEC):
        # sort by (bucket, address): ascending HBM addresses within each
        # bucket call improve DRAM row locality during descriptor drain
        order = np.argsort(b_all[s].astype(np.int64) * 32768 + loc_all[s])
        orders.append(order)
        counts[s] = np.bincount(b_all[s], minlength=NBKT)
    return b_all, loc_all, counts, orders


def _layout(caps):
    """Derived layout from per-(section,bucket) caps [NSEC, NBKT]:
    idx col offsets per R1 call, scratch row offsets, section bases."""
    caps128 = (caps + 127) // 128 * 128
    l1len = caps // 16
    l1off = np.concatenate([[0], np.cumsum(l1len.reshape(-1))]).astype(np.int64)
    r2off = int(l1off[-1])
    tot = r2off + NCHUNK * (CH_IDX // 16)
    S = np.cumsum(caps128, axis=1) - caps128  # within-section row offsets
    scrS = caps128.sum(axis=1)
    sec_base = np.concatenate([[0], np.cumsum(scrS)]).astype(np.int64)
    return caps128, l1off, r2off, tot, S, scrS, sec_base


def _build_inputs(idx, caps):
    """Pack all cores into [128, TOT] int16 idx tiles.
    R1 call m = s*NBKT + b at idx cols [l1off[m], l1off[m+1]);
    R2 chunk j at cols [r2off + j*CH_IDX/16, ...)."""
    caps128, l1off, r2off, tot, S, scrS, sec_base = _layout(caps)
    per_core = []
    for c in range(NCORES):
        idx24 = idx[c * T_CORE : (c + 1) * T_CORE]
        b_all, loc_all, counts, orders = _pack_core(idx24)
        assert (counts <= caps).all()
        tile = np.zeros((P, tot), dtype=np.int16)
        # scratch position of entry e of section s: S[s][bucket] + rank
        scratch_pos = np.zeros((NSEC, SEC_ROWS), dtype=np.int32)
        for s in range(NSEC):
            order = orders[s]
            cnts = counts[s]
            starts = np.concatenate([[0], np.cumsum(cnts)[:-1]])
            rank = np.arange(SEC_ROWS) - starts[b_all[s][order]]
            scratch_pos[s][order] = S[s][b_all[s][order]] + rank
            # R1 lists: per bucket padded to caps[s][b] with its first entry
            sorted_loc = loc_all[s][order]
            for b in range(NBKT):
                cap = int(caps[s, b])
                lst = np.zeros(cap, dtype=np.int16)
                n = cnts[b]
                if n:
                    lst[:n] = sorted_loc[starts[b] : starts[b] + n]
                    lst[n:] = lst[0]
                m = s * NBKT + b
                tile[:, l1off[m] : l1off[m + 1]] = _wrap16(lst)
        # R2 lists: chunk j = s*16 + c covers tokens [j*CH_TOK, (j+1)*CH_TOK);
        # position i: p = i%128, q = i//128; token-in-section u = c*128 + p
        pgrid = np.arange(CH_IDX) % P
        q = np.arange(CH_IDX) // P
        for j in range(NCHUNK):
            s, c = j // (NCHUNK // NSEC), j % (NCHUNK // NSEC)
            u = c * CH_TOK + pgrid
            e = u * K + q
            vals = scratch_pos[s][e].astype(np.int16)
            colbase = r2off + j * (CH_IDX // 16)
            tile[:, colbase : colbase + CH_IDX // 16] = _wrap16(vals)
        per_core.append(tile)
    return per_core


def _gpsimd_schedule():
    """Interleaved gpsimd issue order: ('r1', m) / ('r2', j). R1 is
    section-major (m = s*NBKT + b); R2 of section s-1 interleaves 1:2 into
    R1 of section s after a small lead."""
    sched = []
    for s in range(NSEC):
        r1 = [("r1", s * NBKT + b) for b in range(NBKT)]
        if s == 0:
            sched.extend(r1)
            continue
        r2 = [("r2", (s - 1) * (NCHUNK // NSEC) + c) for c in range(NCHUNK // NSEC)]
        lead, merged, i2 = 4, [], 0
        for i1, item in enumerate(r1):
            merged.append(item)
            if i1 >= lead and i1 % 2 == 0 and i2 < len(r2):
                merged.append(r2[i2])
                i2 += 1
        merged.extend(r2[i2:])
        sched.extend(merged)
    sched.extend(
        ("r2", (NSEC - 1) * (NCHUNK // NSEC) + c) for c in range(NCHUNK // NSEC)
    )
    assert len(sched) == NSEC * NBKT + NCHUNK
    return sched


def _build_nc(caps):
    import contextlib

    import concourse.bacc as bacc
    import concourse.mybir as mybir
    from concourse.library_config import mlp

    caps128, l1off, r2off, tot, S, scrS, sec_base = _layout(caps)
    assert scrS.max() <= 32768
    capc = int(caps128.max()) // P  # staging buffer columns (max call)

    nc = bacc.Bacc("TRN2", num_swdge_queues=NQ)
    W_d = nc.dram_tensor("W", [W_PAD_ROWS, DIM], mybir.dt.float16, kind="ExternalInput")
    idx_d = nc.dram_tensor("idx", [P, tot], mybir.dt.int16, kind="ExternalInput")
    out_d = nc.dram_tensor("out", [T_CORE, DIM], mybir.dt.float16, kind="ExternalOutput")
    scr_d = nc.dram_tensor(
        "scr", [int(sec_base[-1]), DIM], mybir.dt.float16, kind="Internal"
    )

    with (
        nc.Block() as block,
        nc.sbuf_tensor("idx_sb", [P, tot], mybir.dt.int16) as idx_sb,
        nc.semaphore("s_idx") as s_idx,
        nc.semaphore("s_v") as s_v,
        contextlib.ExitStack() as stk,
    ):
        st = [
            stk.enter_context(
                nc.sbuf_tensor(f"st{i}", [P, capc * DIM], mybir.dt.float16)
            )
            for i in range(NBUF)
        ]
        g = [
            stk.enter_context(nc.sbuf_tensor(f"g{i}", [P, K * DIM], mybir.dt.float16))
            for i in range(NBUF)
        ]
        r = [
            stk.enter_context(nc.sbuf_tensor(f"r{i}", [P, DIM], mybir.dt.float16))
            for i in range(2)
        ]
        s_r1q = [stk.enter_context(nc.semaphore(f"s_r1q{i}")) for i in range(NQ)]
        s_g2q = [stk.enter_context(nc.semaphore(f"s_g2q{i}")) for i in range(NQ)]
        # HWDGE store completions are NOT FIFO (16 HW sub-queues), so store
        # sems rotate by buffer lane; "stores x..y all done" becomes exact
        # per-lane counts.
        s_stb = [stk.enter_context(nc.semaphore(f"s_stb{i}")) for i in range(NBUF)]
        s_ostb = [stk.enter_context(nc.semaphore(f"s_ostb{i}")) for i in range(2)]
        sched = _gpsimd_schedule()

        def _lane_count(lane, upto):
            """# of store indices m < upto with m % NBUF == lane."""
            return (upto + NBUF - 1 - lane) // NBUF

        @block.gpsimd
        def _(gpsimd):
            gpsimd.load_library(mlp)
            # split idx preload: section-0 R1 cols first so gathers start early
            split = int(l1off[NBKT])
            gpsimd.dma_start(idx_sb[:, :split], idx_d[:, :split]).then_inc(s_idx, 16)
            gpsimd.dma_start(idx_sb[:, split:], idx_d[:, split:]).then_inc(s_idx, 16)
            gpsimd.wait_ge(s_idx, 16)
            idx_full = False
            for kind, m in sched:
                if not idx_full and not (kind == "r1" and m < NBKT):
                    gpsimd.wait_ge(s_idx, 32)  # rest of idx tile landed
                    idx_full = True
                if kind == "r1":
                    b = m % NBKT
                    nj = int(caps[m // NBKT, b])
                    c128 = int(caps128[m // NBKT, b]) // P
                    if m >= NBUF:
                        # all prior stores on this buffer lane done (incl. m-NBUF)
                        gpsimd.wait_ge(s_stb[m % NBUF], 16 * (m // NBUF))
                    gpsimd.dma_gather(
                        out_ap=st[m % NBUF][:, : c128 * DIM].rearrange(
                            "p (c d) -> p c d", d=DIM
                        ),
                        in_ap=W_d[b * BKT_ROWS : (b + 1) * BKT_ROWS, :],
                        idxs_ap=idx_sb[:, int(l1off[m]) : int(l1off[m]) + nj // 16],
                        num_idxs=nj,
                        num_idxs_reg=nj,
                        elem_size=DIM,
                        queue_num=m % NQ,
                    ).then_inc(s_r1q[m % NQ], 16)
                else:
                    j = m
                    sec = j // (NCHUNK // NSEC)
                    if j % (NCHUNK // NSEC) == 0:
                        # first chunk of the section: section fully stored
                        # (exact per-lane store counts; later chunks of the
                        # section follow in program order, so waits hoist)
                        for lane in range(NBUF):
                            gpsimd.wait_ge(
                                s_stb[lane], 16 * _lane_count(lane, NBKT * (sec + 1))
                            )
                    if j >= NBUF:
                        gpsimd.wait_ge(s_v, 7 * (j - NBUF + 1))
                    cb = r2off + j * (CH_IDX // 16)
                    gpsimd.dma_gather(
                        out_ap=g[j % NBUF][:].rearrange("p (c d) -> p c d", d=DIM),
                        in_ap=scr_d[
                            int(sec_base[sec]) : int(sec_base[sec] + scrS[sec]), :
                        ],
                        idxs_ap=idx_sb[:, cb : cb + CH_IDX // 16],
                        num_idxs=CH_IDX,
                        num_idxs_reg=CH_IDX,
                        elem_size=DIM,
                        queue_num=j % NQ,
                    ).then_inc(s_g2q[j % NQ], 16)
            for i in range(NQ):
                gpsimd.wait_ge(s_g2q[i], 16 * (NCHUNK // NQ))

        @block.sync
        def _(sync):
            for m in range(NSEC * NBKT):
                s, b = m // NBKT, m % NBKT
                sync.wait_ge(s_r1q[m % NQ], 16 * (m // NQ + 1))
                r0w = int(sec_base[s] + S[s, b])
                c128 = int(caps128[s, b])
                sv = scr_d[r0w : r0w + c128, :].rearrange("(c p) d -> p c d", p=P)
                stv = st[m % NBUF][:, : (c128 // P) * DIM].rearrange(
                    "p (c d) -> p c d", d=DIM
                )
                sync.dma_start(sv, stv).then_inc(s_stb[m % NBUF], 16)

        @block.vector
        def _(vector):
            for j in range(NCHUNK):
                vector.wait_ge(s_g2q[j % NQ], 16 * (j // NQ + 1))
                if j >= 2:
                    # all prior out-stores on this r lane done (incl. j-2)
                    vector.wait_ge(s_ostb[j % 2], 16 * (j // 2))
                gs = g[j % NBUF][:].rearrange("p (k d) -> p k d", k=K, d=DIM)
                rs = r[j % 2][:]
                base = (K - 1) * j
                vector.tensor_add(rs, gs[:, 0, :], gs[:, 1, :]).then_inc(s_v, 1)
                for k in range(2, K):
                    vector.wait_ge(s_v, base + k - 1)
                    vector.tensor_add(rs, rs, gs[:, k, :]).then_inc(s_v, 1)

        @block.scalar
        def _(scalar):
            for j in range(NCHUNK):
                scalar.wait_ge(s_v, 7 * (j + 1))
                ov = out_d[j * CH_TOK : (j + 1) * CH_TOK, :]
                scalar.dma_start(ov, r[j % 2][:]).then_inc(s_ostb[j % 2], 16)
            scalar.wait_ge(s_ostb[0], 16 * (NCHUNK // 2))
            scalar.wait_ge(s_ostb[1], 16 * (NCHUNK // 2))

    nc.compile()
    return nc


def _install_trace_hook_if_needed():
    """run_bass_kernel_spmd(trace via BASS_TRACE) under axon needs
    antenv.axon_hooks; the agent image lacks it. Inject a ctypes-based
    equivalent (no-op if a real one is importable). Also make the
    artifact upload failure-proof (no bucket access in the sandbox)."""
    import os

    if not os.environ.get("BASS_TRACE"):
        return
    try:
        from antenv.axon_hooks import get_axon_ntff_profile_hook  # noqa: F401

        _has = get_axon_ntff_profile_hook() is not None
    except ImportError:
        _has = False
    if not _has:
        import contextlib
        import ctypes
        import types

        so = "/opt/axon/libaxon_pjrt.so"
        if os.path.exists(so):
            lib = ctypes.CDLL(so)
            if hasattr(lib, "axon_start_nrt_profile"):
                lib.axon_start_nrt_profile.argtypes = [
                    ctypes.POINTER(ctypes.c_int64),
                    ctypes.c_size_t,
                ]
                lib.axon_start_nrt_profile.restype = ctypes.c_int64
                lib.axon_stop_nrt_profile.argtypes = [ctypes.c_char_p]
                lib.axon_stop_nrt_profile.restype = ctypes.c_int64

                @contextlib.contextmanager
                def _hook(output_dir, device_ids):
                    import jax

                    jax.devices()
                    if device_ids:
                        ids = (ctypes.c_int64 * len(device_ids))(*device_ids)
                        rc = lib.axon_start_nrt_profile(ids, len(device_ids))
                    else:
                        rc = lib.axon_start_nrt_profile(None, 0)
                    if rc != 0:
                        raise RuntimeError(f"axon_start_nrt_profile rc={rc}")
                    try:
                        yield
                    finally:
                        n = lib.axon_stop_nrt_profile(str(output_dir).encode())
                        print(
                            f"ntff profile: {n} files -> {output_dir}",
                            file=sys.stderr,
                        )

                mod = types.ModuleType("antenv.axon_hooks")
                mod.get_axon_ntff_profile_hook = lambda: _hook
                mod.set_axon_ntff_profile_hook = lambda h: None
                sys.modules["antenv.axon_hooks"] = mod

    import concourse.bass_utils as bu

    if not getattr(bu.upload_artifacts, "_safe_wrapped", False):
        _orig = bu.upload_artifacts

        def _safe_upload(tmpdir):
            try:
                return _orig(tmpdir)
            except Exception:
                return f"file://{tmpdir}"

        _safe_upload._safe_wrapped = True
        bu.upload_artifacts = _safe_upload


def kernel(t, W):
    t = np.asarray(t, dtype=np.int64)
    W = np.asarray(W, dtype=np.float32)
    assert t.shape == (B, S) and W.shape == (NUM, DIM)

    r = np.arange(K, dtype=np.int64)
    h = _mueller_hash(t.reshape(-1)[:, None] + r[None, :])
    idx = (h % NUM).astype(np.int32)  # [T, K] in [0, NUM)

    # per-(section,bucket) caps, uniform across cores (program shape baked)
    allb = (idx.reshape(NCORES, NSEC, SEC_ROWS) >> 15).astype(np.int32)
    cnts = np.zeros((NCORES, NSEC, NBKT), dtype=np.int64)
    for c in range(NCORES):
        for s in range(NSEC):
            cnts[c, s] = np.bincount(allb[c, s], minlength=NBKT)
    caps = (cnts.max(axis=0) + 15) // 16 * 16  # [NSEC, NBKT], 16-granular
    assert caps.max() <= CH_IDX
    assert ((caps + 127) // 128 * 128).sum(axis=1).max() <= 32768

    per_core = _build_inputs(idx, caps)

    Wq = np.zeros((W_PAD_ROWS, DIM), dtype=np.float16)
    Wq[:NUM] = (W * np.float32(0.125)).astype(np.float16)

    _install_trace_hook_if_needed()
    from concourse.bass_utils import run_bass_kernel_spmd

    key = ("nc", caps.tobytes())
    if key not in _NC_CACHE:
        _NC_CACHE[key] = _build_nc(caps)
    nc = _NC_CACHE[key]

    in_maps = [{"W": Wq, "idx": per_core[c]} for c in range(NCORES)]
    core_ids = list(range(NCORES))
    try:
        res = run_bass_kernel_spmd(nc, in_maps, core_ids)
    except Exception as e:  # one retry for transient device/runtime hiccups
        print(f"run_bass_kernel_spmd failed ({e!r}); retrying once", file=sys.stderr)
        res = run_bass_kernel_spmd(nc, in_maps, core_ids)
    if res.exec_time_ns is not None:
        print(
            f"kernel exec_time_ns={res.exec_time_ns} "
            f"mean={res.mean_exec_time_ns}",
            file=sys.stderr,
        )
    _NC_CACHE["last_result"] = res

    out = np.concatenate(
        [res.results[c]["out"].astype(np.float32) for c in range(NCORES)], axis=0
    )
    return out.reshape(B, S, DIM)
